# revision 35
# baseline (speedup 1.0000x reference)
"""Trainium2 Bass kernel for the MoE-routing module.

Computation (B=32768, D=1024, H=512, F=100, E=16, K=2):
    h   = relu(x @ W_shared + b_shared)                  [B, H]
    a   = relu(einsum('bh,ehf', h, W1) + b1)             [B, E, F]
    o   = einsum('bef,efo', a, W2) + b2                  [B, E, 1]
    out = mean over the K routed experts of o[b, send_to[idx[b]]]

Strategy: host sorts tokens by head id and shards the sorted batch over
the 8 cores (4096 tokens each), 9 device chunks per core (two 256-token
head chunks let M1 start while the DMA backlog clears).  A head group
covers ~4 chunks, so most chunk positions hold a single head id on every
core: those need exactly the 2 routed experts, and the top-2 mean
collapses to a constant 0.5/0.5 blend folded into W2 — the select stage
merges into M3 as a 1-column matmul (no mask, no vector work).
Positions where any core crosses a head boundary run a general masked
path with EC slots (3 normally).  Per-position structure (slot count +
masked?) is uniform across cores, so one SPMD program serves all 8;
programs are cached per structure key.

All matmuls run in fp16: same 1 cycle/row PE rate as fp32r at 512-wide
moving tiles, but half the HBM traffic; final rel err ~6e-4 (fp8 was
measured at 4e-2 — over the 2e-2 budget — and is not used).

Stages (features on SBUF partitions throughout):
  M1: hT[h, t]  = relu(W_shared.T @ xT + b)       8 k-tiles as 2 halves
  M2: aT[f', t] = relu(W1sel.T @ hT + b1)         f' = slot*128 + f
  M3 single: out[t] = 0.5*(W2cat).T @ aT + b2m    1-col lhsT, merged sel
  M3 mixed:  c[j, t] = W2bd.T @ aT; out = ones.T @ (c * mask) + b2m[t]

Schedule notes (each worth measured microseconds on the NTFF trace):
- the PE sequencer does NOT push past an unsatisfied semaphore wait, so
  any cross-engine dependency (psum bank free, act output ready) parks
  it, drains the 64-deep PE queue, and exposes the next LDWEIGHTS as a
  ~100ns stall on the following matmul.  The whole schedule is built to
  keep those waits pre-satisfied: M1 runs as two interleaved m-pair
  psum groups (one group boundary per 16 MMs); chunk N's M2 is emitted
  between chunk N+1's m-pair groups; chunk N's M3 two chunks later; a
  mixed chunk's ones-matmul select three chunks later (its DVE mask-
  multiply is then long done);
- ALL DMAs ride the single sync (SP) HWDGE ring: one InstDMACopy
  already spreads across all 16 SDMA engines (full ~358 GB/s), a
  single FIFO gives exact need-time ordering with no inter-ring
  jitter, and the Act queue never stalls behind a DMA wait;
- dummy PE matmuls bridge the ~5us DMA pipe ramp and keep the HAM
  clock-gate warming, so real M1 runs at 2.4 GHz from the start;
- M3 single-head slot matmuls are column-tiled (tile_position) into
  different PE column groups writing different psum partitions; the
  cross-slot sum rides the otherwise-idle DVE;
- 512-token singles lead the order (the head is gated by the W_shared
  halves either way), mixed chunks run late (their x would starve the
  DMA-tight head), and a 256-token single closes the program so the
  final M1->M2->M3 tail chain is short;
- chunk outputs accumulate in one SBUF row; a single batched out DMA
  replaces 9 tiny ones (each tiny DMA pays a ~2us HBM write receipt).

Rejected on measurement: fp8 anywhere (e4m3 DoubleRow would double M1
throughput, but one fp8 stage costs 3.5-4.7e-2 max-rel-err vs the 2e-2
budget, and any error-compensation scheme needs >=2 passes, erasing the
2x win); quarter-granularity x streaming (descriptor overhead slowed
the aggregate stream and oscillated HAM).
"""

import numpy as np

import concourse.mybir as mybir
from concourse import bacc
from concourse.bass_utils import run_bass_kernel_spmd
from concourse.tile import TileContext

B, D, H, F, E, TOPK = 32768, 1024, 512, 100, 16, 2
N_CORES = 8
BL = B // N_CORES          # tokens per core
CHUNK = 512                # max tokens per device-side tile loop
# smaller head chunks let M1 start while the DMA backlog clears
SIZES = (256, 256, 512, 512, 512, 512, 512, 512, 512)
OFFS = [0]
for _s in SIZES:
    OFFS.append(OFFS[-1] + _s)
assert OFFS[-1] == BL
NCH = len(SIZES)           # chunks per core
KD = D // 128              # M1 contraction tiles
NPAIR = KD // 2            # M1 contraction tile pairs (DMA granularity)
MH = H // 128              # M1 output tiles
KH = H // 128              # M2 contraction tiles

COMPUTE_DT = "float16"
CDT = mybir.dt.float16
NP_CDT = np.float16
_FP32 = mybir.dt.float32
_cache = {}


def _build_nc(key):
    """Build the SPMD program for per-position (slot count, masked) key."""
    ecs, mixed = key
    n_mixed = sum(mixed)
    max_ec = max(ecs)
    MROWS = 33                           # mask rows: slots + b2mean at row 32
    W2COLS = sum(e * e if mx else e for e, mx in zip(ecs, mixed))
    NB = MH + sum(ecs) + sum(0 if mx else 1 for mx in mixed)

    nc = bacc.Bacc("TRN2", target_bir_lowering=False, num_devices=N_CORES)

    xT_d = nc.declare_dram_parameter("xT", [D * BL], CDT, isOutput=False)
    wsh_d = nc.declare_dram_parameter("wsh", [D * H], CDT, isOutput=False)
    w1sz = [KH * 128 * e * 128 for e in ecs]
    w1off = np.cumsum([0] + w1sz).tolist()
    w1c_d = nc.declare_dram_parameter("w1c", [w1off[-1]], CDT, isOutput=False)
    w2_d = nc.declare_dram_parameter("w2", [128, W2COLS], CDT, isOutput=False)
    bias_d = nc.declare_dram_parameter("biases", [128, NB], _FP32, isOutput=False)
    bias2_d = nc.declare_dram_parameter("bias2", [1, BL], _FP32, isOutput=False)
    if n_mixed:
        mask_d = nc.declare_dram_parameter(
            "mask", [MROWS, n_mixed * CHUNK], _FP32, isOutput=False
        )
    out_d = nc.declare_dram_parameter("out", [BL], _FP32, isOutput=True)

    relu = mybir.ActivationFunctionType.Relu

    with TileContext(nc) as tc:
        with (
            tc.tile_pool(name="weights", bufs=1) as wpool,
            tc.tile_pool(name="xin", bufs=24) as xpool,
            tc.tile_pool(name="w1p", bufs=3) as w1pool,
            tc.tile_pool(name="hmid", bufs=3) as hpool,
            tc.tile_pool(name="amid", bufs=3) as apool,
            tc.tile_pool(name="small", bufs=10) as spool,
            tc.tile_pool(name="ps_h", bufs=4, space="PSUM") as ps_h,
            tc.tile_pool(name="ps_a", bufs=2, space="PSUM") as ps_a,
            tc.tile_pool(name="ps_c", bufs=1, space="PSUM") as ps_c,
            tc.tile_pool(name="ps_o", bufs=1, space="PSUM") as ps_o,
        ):
            # ---- DMAs with explicit priorities pinning queue order ----
            _prio = [0]

            def pdma(q, dst, src):
                inst = q.dma_start(dst, src)
                inst.ins.bass_priority = _prio[0]
                _prio[0] += 1
                return inst

            def xhalf_view(j, h):
                sz = SIZES[j]
                o = (OFFS[j] * D) + h * (128 * 4 * sz)
                return xT_d[o : o + 128 * 4 * sz].rearrange(
                    "(p q t) -> p q t", p=128, q=4
                )

            def wsh_view(h):
                o = h * (128 * 4 * H)
                return wsh_d[o : o + 128 * 4 * H].rearrange(
                    "(p q h) -> p q h", p=128, q=4
                )

            def w1_view(j):
                return w1c_d[w1off[j] : w1off[j + 1]].rearrange(
                    "(p k c) -> p k c", p=128, k=KH
                )

            # Chunk processing order: 512-token singles lead (the head is
            # gated by the W_shared halves, so a big first chunk costs
            # nothing), mixed (masked) chunks run late — their x would
            # starve the DMA-tight head — but not last, and a small single
            # closes the program so the final M2/M3 tail chain is short.
            singles = [j for j in range(NCH) if not mixed[j]]
            mixed_js = [j for j in range(NCH) if mixed[j]]
            small_singles = [j for j in singles if SIZES[j] < CHUNK]
            if len(singles) >= 3:
                tail = [small_singles[-1]] if small_singles else [singles[-1]]
                pool = [j for j in singles if j not in tail]
                order = pool[:-1] + mixed_js + pool[-1:] + tail
            else:
                order = singles + mixed_js

            # head: wsh halves + first-processed chunk's x halves, split
            # across both HW rings (4KB per-partition runs, few issues)
            wshh = [wpool.tile([128, 4, H], CDT, name=f"wshh{h}") for h in range(2)]
            xts = [
                [
                    xpool.tile([128, 4, SIZES[j]], CDT, tag="xt", name=f"xt{j}_{h}")
                    for h in range(2)
                ]
                for j in range(NCH)
            ]
            # ALL DMAs ride the sync (SP) HWDGE ring: one InstDMACopy already
            # spreads across all 16 SDMA engines (full HBM bandwidth), a
            # single FIFO gives exact need-time ordering with no inter-ring
            # jitter, the Act queue never stalls on DMA waits, and unused
            # scalar/gpsimd DMA paths shorten the end-of-program quiesce.
            j0 = order[0]
            j1 = order[1]
            bias_sb = wpool.tile([128, NB], _FP32)
            bias2_sb = wpool.tile([1, BL], _FP32)
            w2_sb = wpool.tile([128, W2COLS], CDT)
            if n_mixed:
                mask_sb = wpool.tile([MROWS, n_mixed * CHUNK], _FP32)
                ones_sb = wpool.tile([max_ec, 1], CDT)
                nc.vector.memset(ones_sb[:], 1.0)
            w1sb = [None] * NCH
            for j in (j0, j1):
                w1sb[j] = w1pool.tile(
                    [128, KH, ecs[j] * 128], CDT, tag="w1", name=f"w1_{j}"
                )

            # need-time order: first-chunk M1 inputs, second-chunk M1 inputs
            # interleaved with the small bias/select tensors, then w1s.
            # The first (wsh, x) pair is split into interleaved halves so the
            # k-outer loop's first matmuls start one transfer earlier.
            for h in range(2):
                pdma(nc.sync, wshh[h][:, 0:2, :], wsh_view(h)[:, 0:2, :])
                pdma(
                    nc.sync, xts[j0][h][:, 0:2, :], xhalf_view(j0, h)[:, 0:2, :]
                )
                pdma(nc.sync, wshh[h][:, 2:4, :], wsh_view(h)[:, 2:4, :])
                pdma(
                    nc.sync, xts[j0][h][:, 2:4, :], xhalf_view(j0, h)[:, 2:4, :]
                )
            pdma(nc.sync, xts[j1][0][:], xhalf_view(j1, 0))
            pdma(nc.sync, bias_sb[:], bias_d[:])
            pdma(nc.sync, w1sb[j0][:], w1_view(j0))
            pdma(nc.sync, xts[j1][1][:], xhalf_view(j1, 1))
            pdma(nc.sync, bias2_sb[:], bias2_d[:])
            pdma(nc.sync, w2_sb[:], w2_d[:])
            pdma(nc.sync, w1sb[j1][:], w1_view(j1))
            if n_mixed:
                pdma(nc.sync, mask_sb[:], mask_d[:])

            def fetch_chunk(j, oi):
                nc.sync.dma_start(xts[j][0][:], xhalf_view(j, 0))
                nc.sync.dma_start(xts[j][1][:], xhalf_view(j, 1))
                w1sb[j] = w1pool.tile(
                    [128, KH, ecs[j] * 128], CDT, tag="w1", name=f"w1_{j}"
                )
                nc.sync.dma_start(w1sb[j][:], w1_view(j))

            # out accumulator: chunks write their [1, sz] results here; one
            # batched sync-ring DMA ships the whole core's output at the end
            # (9 tiny SWDGE outs each paid ~2us completion receipts).
            out_acc = wpool.tile([1, BL], _FP32)

            # ---- PE warm-up: back-to-back dummy matmuls keep the HAM
            # clock-gate ramping to full speed while the first DMAs land,
            # so real M1 runs at 2.4 GHz from the start.
            warm_sb = wpool.tile([128, 512], CDT)
            nc.vector.memset(warm_sb[:], 0.0)
            # dummy activation: forces the Act engine's ~1.3us relu
            # table load into the DMA-ramp dead time (otherwise it sits
            # in front of the first real M1 activation and stalls M2)
            warm_act = wpool.tile([1, 16], _FP32)
            nc.scalar.activation(warm_act[:], warm_sb[0:1, 0:16], relu)
            warm_ps = ps_a.tile([128, 512], _FP32, tag="ps_a", name="warm_ps")
            for wi in range(9):
                nc.tensor.matmul(
                    warm_ps[:],
                    lhsT=warm_sb[:, :128],
                    rhs=warm_sb[:],
                    start=True,
                    stop=True,
                )

            # ---- compute ----
            # per-position col offsets into w2_sb / bias_sb / mask (host
            # packs in position order)
            w2cs, bcols, mjs = [], [], []
            _w2c, _bcol, _mj = 0, MH, 0
            for j in range(NCH):
                w2cs.append(_w2c)
                bcols.append(_bcol)
                mjs.append(_mj)
                if mixed[j]:
                    _w2c += ecs[j] * ecs[j]
                    _bcol += ecs[j]
                    _mj += 1
                else:
                    _w2c += ecs[j]
                    _bcol += ecs[j] + 1

            hTs = {}

            def emit_sel(sb):
                """Mixed-path stage B: ones-matmul select + bias add.  Runs
                one chunk after stage A so the DVE mask-multiply is long done
                when the PE reaches the ones-matmul — no cross-engine stall."""
                j = sb["j"]
                ec = ecs[j]
                sz = SIZES[j]
                mj = mjs[j]
                t0 = OFFS[j]
                ot = out_acc[0:1, t0 : t0 + sz]
                po = ps_o.tile([1, sz], _FP32, tag="ps_o", name=f"pom{j}")
                nc.tensor.matmul(
                    po[:], lhsT=ones_sb[:ec, :], rhs=sb["msel"][:],
                    start=True, stop=True,
                )
                nc.vector.tensor_add(
                    ot, po[:],
                    mask_sb[32:33, mj * CHUNK : mj * CHUNK + sz],
                )

            aTs = {}

            def emit_m2(j):
                """M2 for chunk j, emitted inside the NEXT chunk's M1 gap:
                the sequencer reaches M2's act-semaphore waits while the PE
                queue still holds ~16 M1 matmuls, so the waits clear before
                the queue drains and the LDWEIGHTS prefetch cleanly."""
                ec = ecs[j]
                sz = SIZES[j]
                bcol = bcols[j]
                hT = hTs[j]

                # M2: aT = relu(W1sel.T @ hT + b1)
                aT = apool.tile([128, ec, sz], CDT, tag="aT", name=f"aT{j}")
                aTs[j] = aT
                if ec == 2:
                    # both slots' accumulations interleaved: one group
                    # boundary for 8 MMs
                    pas = [
                        ps_a.tile([128, sz], _FP32, tag="ps_a", name=f"pa{j}_{mi}")
                        for mi in range(2)
                    ]
                    for k in range(KH):
                        for mi in range(2):
                            nc.tensor.matmul(
                                pas[mi][:],
                                lhsT=w1sb[j][:, k, mi * 128 : (mi + 1) * 128],
                                rhs=hT[:, k, :],
                                start=(k == 0),
                                stop=(k == KH - 1),
                            )
                    for mi in range(2):
                        nc.scalar.activation(
                            aT[:, mi, :], pas[mi][:], relu,
                            bias=bias_sb[:, bcol + mi : bcol + mi + 1],
                        )
                else:
                    for mi in range(ec):
                        pa = ps_a.tile(
                            [128, sz], _FP32, tag="ps_a", name=f"pa{j}_{mi}"
                        )
                        for k in range(KH):
                            nc.tensor.matmul(
                                pa[:],
                                lhsT=w1sb[j][:, k, mi * 128 : (mi + 1) * 128],
                                rhs=hT[:, k, :],
                                start=(k == 0),
                                stop=(k == KH - 1),
                            )
                        nc.scalar.activation(
                            aT[:, mi, :], pa[:], relu,
                            bias=bias_sb[:, bcol + mi : bcol + mi + 1],
                        )

            def emit_m3(j, last):
                """M3 + select for chunk j, emitted TWO chunks later: its aT
                activations are then ~9us old, so no semaphore park drains
                the PE queue in front of the 1-column LDWEIGHTS."""
                ec = ecs[j]
                sz = SIZES[j]
                w2c = w2cs[j]
                mj = mjs[j]
                aT = aTs[j]
                t0 = OFFS[j]
                ot = out_acc[0:1, t0 : t0 + sz]
                if not mixed[j]:
                    if last:
                        # last chunk: psum-accumulated form keeps the output
                        # chain shortest (1 DVE op after the matmuls).
                        # Allocate from the otherwise-idle ps_c pool: ps_o's
                        # single buffer is held by the previous chunk's slow
                        # serial DVE add chain (~1.4us) and would stall these
                        # matmuls at the very end of the program.
                        po = ps_c.tile([1, sz], _FP32, tag="ps_c", name=f"po{j}")
                        for k in range(ec):
                            nc.tensor.matmul(
                                po[:],
                                lhsT=w2_sb[:, w2c + k : w2c + k + 1],
                                rhs=aT[:, k, :],
                                start=(k == 0),
                                stop=(k == ec - 1),
                            )
                        nc.vector.tensor_add(
                            ot, po[:], bias2_sb[0:1, t0 : t0 + sz]
                        )
                    else:
                        # col-tiled: slot matmuls land in different PE column
                        # groups and run concurrently instead of serially;
                        # the cross-row sum moves to the idle DVE
                        po = ps_o.tile(
                            [32 * ec + 1, sz], _FP32, tag="ps_o", name=f"po{j}"
                        )
                        for k in range(ec):
                            nc.tensor.matmul(
                                po[32 * k : 32 * k + 1, :],
                                lhsT=w2_sb[:, w2c + k : w2c + k + 1],
                                rhs=aT[:, k, :],
                                start=True,
                                stop=True,
                                tile_position=(0, 32 * k),
                            )
                        # DVE reads at most one PSUM operand per op: chain
                        # psum-row adds through the sbuf accumulator
                        nc.vector.tensor_add(
                            ot, po[0:1, :], bias2_sb[0:1, t0 : t0 + sz]
                        )
                        for k in range(1, ec):
                            nc.vector.tensor_add(ot, po[32 * k : 32 * k + 1, :], ot)
                else:
                    pc = ps_c.tile([ec, sz], _FP32, tag="ps_c", name=f"pc{j}")
                    for k in range(ec):
                        nc.tensor.matmul(
                            pc[:],
                            lhsT=w2_sb[:, w2c + k * ec : w2c + (k + 1) * ec],
                            rhs=aT[:, k, :],
                            start=(k == 0),
                            stop=(k == ec - 1),
                        )
                    msel = spool.tile([ec, sz], CDT, tag="msel", name=f"msel{j}")
                    nc.vector.tensor_mul(
                        msel[:], pc[:],
                        mask_sb[:ec, mj * CHUNK : mj * CHUNK + sz],
                    )
                    sb = {"j": j, "msel": msel}
                    if last:
                        emit_sel(sb)
                    else:
                        return sb
                return None

            pend_m2 = None
            pend_m3 = None
            pend_sel = None
            for oi in range(NCH):
                j = order[oi]
                sz = SIZES[j]
                xt = xts[j]
                if oi + 2 < NCH:
                    fetch_chunk(order[oi + 2], oi)

                # M1: hT = relu(W_shared.T @ xT + b)
                hT = hpool.tile([128, MH, sz], CDT, tag="hT", name=f"hT{j}")
                hTs[j] = hT
                # m-pairs interleaved (one psum-group boundary per 16 MMs,
                # only 2 ps_h banks held at a time); the (p, i)-outer order
                # consumes the head's quarter DMAs as they land, so the
                # first chunk starts as early as the old k-outer form did.
                # The previous chunks' M2/M3 are emitted between the two
                # pair groups so their semaphore waits clear while the PE
                # queue is still full of M1 work.
                for mp in range(MH // 2):
                    phs2 = [
                        ps_h.tile(
                            [128, sz], _FP32, tag="ps_h", name=f"ph{j}_{m}"
                        )
                        for m in (2 * mp, 2 * mp + 1)
                    ]
                    for p in range(NPAIR):
                        for i in range(2):
                            h, q = p // 2, (p % 2) * 2 + i
                            for mi, m in enumerate((2 * mp, 2 * mp + 1)):
                                nc.tensor.matmul(
                                    phs2[mi][:],
                                    lhsT=wshh[h][:, q, m * 128 : (m + 1) * 128],
                                    rhs=xt[h][:, q, :],
                                    start=(p == 0 and i == 0),
                                    stop=(p == NPAIR - 1 and i == 1),
                                )
                    for mi, m in enumerate((2 * mp, 2 * mp + 1)):
                        nc.scalar.activation(
                            hT[:, m, :], phs2[mi][:], relu,
                            bias=bias_sb[:, m : m + 1],
                        )
                    if mp == 0:
                        if pend_sel is not None:
                            emit_sel(pend_sel)
                            pend_sel = None
                        if pend_m3 is not None:
                            pend_sel = emit_m3(pend_m3, False)
                            pend_m3 = None
                        if pend_m2 is not None:
                            emit_m2(pend_m2)
                            pend_m3 = pend_m2
                            pend_m2 = None
                pend_m2 = j

            # flush: last chunk L's M2 first (its aT acts start immediately
            # after L's M1 acts), the older chunk's M3 next (deps ancient —
            # fills the PE while L's acts land), then L's M3 with its
            # act-waits pre-satisfied
            if pend_sel is not None:
                emit_sel(pend_sel)
            emit_m2(pend_m2)
            if pend_m3 is not None:
                sb = emit_m3(pend_m3, False)
                if sb is not None:
                    emit_sel(sb)
            emit_m3(pend_m2, True)

            # single batched output DMA on the sync ring
            nc.sync.dma_start(
                out_d[:].rearrange("(o t) -> o t", o=1), out_acc[:]
            )

    nc.compile()
    return nc


def get_nc(key):
    if key not in _cache:
        _cache[key] = _build_nc(key)
    return _cache[key]


def prepare(inputs):
    """Host-side routing/sorting/sharding. Returns (key, in_maps, perm)."""
    x = np.asarray(inputs["x"], dtype=np.float32)
    idx = np.asarray(inputs["idx"]).astype(np.int64).reshape(B)
    W_shared = np.asarray(inputs["W_shared"], dtype=np.float32)
    b_shared = np.asarray(inputs["b_shared"], dtype=np.float32).reshape(H)
    W1 = np.asarray(inputs["W1"], dtype=np.float32)
    b1 = np.asarray(inputs["b1"], dtype=np.float32).reshape(E, F)
    W2 = np.asarray(inputs["W2"], dtype=np.float32).reshape(E, F)
    b2 = np.asarray(inputs["b2"], dtype=np.float32).reshape(E)
    send_to = np.asarray(inputs["send_to"]).astype(np.int64)

    perm = np.argsort(idx, kind="stable")
    idx_s = idx[perm]
    x_s = x[perm]
    routes_s = send_to[idx_s]                      # [B, K] sorted routes

    # per-position structure: slot count + masked?, uniform across cores
    slot_lists = [[None] * NCH for _ in range(N_CORES)]
    ecs, mixed = [], []
    for j in range(NCH):
        ec_j, mx_j = 2, False
        for c in range(N_CORES):
            sl = slice(c * BL + OFFS[j], c * BL + OFFS[j + 1])
            experts = np.unique(routes_s[sl])
            slot_lists[c][j] = experts
            ec_j = max(ec_j, len(experts))
            if len(np.unique(idx_s[sl])) > 1:
                mx_j = True
        ecs.append(ec_j)
        mixed.append(mx_j)
    ecs, mixed = tuple(ecs), tuple(mixed)
    n_mixed = sum(mixed)
    max_ec = max(ecs)
    MROWS = 33

    # wsh half blocks [half, 128, 4, H]
    wshr = W_shared.reshape(2, 4, 128, H)
    wsh_flat = np.ascontiguousarray(wshr.transpose(0, 2, 1, 3)).astype(NP_CDT).ravel()

    in_maps = []
    for c in range(N_CORES):
        xc = x_s[c * BL : (c + 1) * BL]
        # per-chunk half blocks [half, 128, 4, sz]
        xparts = []
        for j in range(NCH):
            xj = xc[OFFS[j] : OFFS[j + 1]].reshape(SIZES[j], 2, 4, 128)
            xparts.append(
                np.ascontiguousarray(xj.transpose(1, 3, 2, 0)).astype(NP_CDT).ravel()
            )
        xT = np.concatenate(xparts)

        w1_parts = []
        w2_cols = []
        bias_cols = [b_shared.reshape(MH, 128).T]
        bias2_row = np.zeros((1, BL), np.float32)
        mask_cols = np.zeros((MROWS, max(n_mixed, 1) * CHUNK), np.float32)
        mj = 0
        for j in range(NCH):
            sl = slice(c * BL + OFFS[j], c * BL + OFFS[j + 1])
            ec = ecs[j]
            slots = np.full(ec, -1, dtype=np.int64)
            el = slot_lists[c][j]
            slots[: len(el)] = el

            w1sel = np.zeros((H, ec * 128), np.float32)
            b1sel = np.zeros(ec * 128, np.float32)
            for mi, e in enumerate(slots):
                if e < 0:
                    continue
                w1sel[:, mi * 128 : mi * 128 + F] = W1[e]
                b1sel[mi * 128 : mi * 128 + F] = b1[e]
            w1_parts.append(
                np.ascontiguousarray(
                    w1sel.reshape(KH, 128, ec * 128).transpose(1, 0, 2)
                ).astype(NP_CDT).ravel()
            )
            bias_cols.append(b1sel.reshape(ec, 128).T)

            r = routes_s[sl]                        # [CHUNK, K]
            if not mixed[j]:
                w2m = np.zeros((128, ec), np.float32)
                for e in r[0]:  # routes with multiplicity
                    mi = int(np.where(slots == e)[0][0])
                    w2m[:F, mi] += W2[e] / r.shape[1]
                w2_cols.append(w2m)
                col = np.zeros((128, 1), np.float32)
                col[0, 0] = b2[r[0]].mean()
                bias_cols.append(col)
                bias2_row[0, OFFS[j] : OFFS[j + 1]] = b2[r[0]].mean()
            else:
                w2full = np.zeros((ec * 128, ec), np.float32)
                for mi, e in enumerate(slots):
                    if e < 0:
                        continue
                    w2full[mi * 128 : mi * 128 + F, mi] = W2[e]
                w2_cols.append(
                    w2full.reshape(ec, 128, ec).transpose(1, 0, 2).reshape(128, ec * ec)
                )
                sz = SIZES[j]
                for k in range(r.shape[1]):
                    hit = slots[:, None] == r[None, :, k]
                    mask_cols[:ec, mj * CHUNK : mj * CHUNK + sz] += (
                        hit.astype(np.float32) / r.shape[1]
                    )
                mask_cols[32, mj * CHUNK : mj * CHUNK + sz] = b2[r].mean(axis=1)
                mj += 1

        in_map = {
            "xT": xT,
            "wsh": wsh_flat,
            "w1c": np.concatenate(w1_parts),
            "w2": np.concatenate(w2_cols, axis=1).astype(NP_CDT),
            "biases": np.ascontiguousarray(
                np.concatenate(bias_cols, axis=1)
            ).astype(np.float32),
            "bias2": bias2_row,
        }
        if n_mixed:
            in_map["mask"] = mask_cols
        in_maps.append(in_map)
    return (ecs, mixed), in_maps, perm


def kernel(**inputs) -> np.ndarray:
    key, in_maps, perm = prepare(inputs)
    nc = get_nc(key)
    res = run_bass_kernel_spmd(nc, in_maps, list(range(N_CORES)))
    out_sorted = np.concatenate([res.results[c]["out"] for c in range(N_CORES)])
    out = np.empty(B, dtype=np.float32)
    out[perm] = out_sorted
    return out.reshape(B, 1)

